# revision 13
# baseline (speedup 1.0000x reference)
"""Trainium2 Bass kernel for Conf-MPU loss (nn_Conf_MPULoss) — v3.

Host side: rows sorted by label t into 5 class groups, split evenly across 8
cores, each per-core class segment padded to S = 128*R rows with sentinel rows
(classes 0..3 = -10, class 4 = +10; exact in bf16). x ships as bf16 in a
PLANAR layout per segment: [P, 6 planes, R] where planes 0..4 = x[:, j] and
plane 5 is a HOLE the device fills with -x_c (so one big ScalarE exp also
yields exp(-x_c) = 1/e_c — no divide ALU op exists on any engine). Per-class
counts come from a host bincount; the C-length accumulators from all cores
are host all-reduced and combined into the final scalar.

Device per class segment c (planes as [P, R] bf16 slices):
    X5   = -X_c                     DVE tensor_scalar (4x mode)
    E    = exp(X[:, :6R])           ScalarE, ONE instr (plane5 -> 1/e_c)
    Z    = ((E0+E1)+(E2+E3))+E4     A-pairs on GpSimd TT, rest DVE TT (2x)
    lnZ  = ln(Z)                    ScalarE
    d4   = lnZ - X4                 DVE TT (= -log p_neg)
    c < 4:
      sd_c accum: sum over planes(4,5) = sum(x4 - x_c)   DVE tensor_scalar 4x
           (risk1-risk3 needs only this: the per-row lnZ terms cancel)
      m = (2*E_c > Z) (== p_c>.5)   DVE STT, fused accum -> den_c
      q = d4 * Z ; u = q * E5       DVE TT / GpSimd TT  (u = -log(p4)/p_c)
      num_c accum: sum(m * u)       DVE STT, fused accum
    c == 4:
      MX = max tree over E planes   M-pairs on GpSimd, rest DVE (2x)
      mn = (2*MX <= Z)              DVE STT (== all p <= .5)
      li accum: sum(mn * d4)        DVE STT, fused accum

Emission is software-pipelined across the 5 independent segments (seg4 first
and in half-chunks to prime the pipe, seg3 last in half-chunks to shorten the
dependency tail). Accumulator columns are unique per (segment, chunk); the
host sums them. Budgets per core: ScalarE ~22us (29R exp + 5R ln), DVE ~21us,
GpSimd ~18us, DMA ~15us (5.0MB bf16).

Pad rows give m=mn=0, d4=0 and an exact +20.0 per pad row in sd, corrected
on host. exp without max-subtraction is bf16-safe: logits are O(1).
"""

import ml_dtypes
import numpy as np

import concourse.bacc as bacc
import concourse.mybir as mybir
import concourse.tile as tile
from concourse import bass_utils

F32 = mybir.dt.float32
BF16 = mybir.dt.bfloat16
Alu = mybir.AluOpType
Act = mybir.ActivationFunctionType

P = 128
NCLS = 5
N_CORES = 8
# stat columns: per class c<4: base 8c + {sd:0,1  den:2,3  num:4,5}; li: 32,33
NSTAT = 34

PAD_POS = -10.0
PAD_NEG = 10.0

_PROGRAM_CACHE: dict[int, tuple] = {}


def _restrict_act_tables(arch: str):
    """Confine Exp/Ln to the natural_log_exp_and_others set so the act-table
    pass emits a single ACT_TABLE_LOAD instead of thrashing between the
    exp_and_others and natural_log sets (~1.3us per load)."""
    from concourse import hw_specs

    tables = hw_specs.get_activation_tables(arch)
    if "natural_log_exp_and_others" not in tables:
        return
    for name, funcs in tables.items():
        if name != "natural_log_exp_and_others":
            funcs.discard(Act.Exp)
            funcs.discard(Act.Ln)


def _build_program(R: int):
    """Build + compile the per-core Bass program for segment length S=128*R."""
    nc = bacc.Bacc("TRN2", debug=False, num_devices=N_CORES)
    _restrict_act_tables(nc.m.arch)
    # every segment region is 6 planes wide in DRAM; plane 5 is junk for c==4
    x_d = nc.dram_tensor("x", [NCLS, P, 6 * R], BF16, kind="ExternalInput").ap()
    st_d = nc.dram_tensor("stats", [P, NSTAT], F32, kind="ExternalOutput").ap()

    with tile.TileContext(nc) as tc:
        with (
            tc.tile_pool(name="io", bufs=1) as iop,
            tc.tile_pool(name="ep", bufs=1) as epool,
            tc.tile_pool(name="wk", bufs=1) as wp,
            tc.tile_pool(name="st", bufs=1) as sp,
        ):
            stats = sp.tile([P, NSTAT], F32)
            X = {}
            E = {}
            seg = {}

            def dma_in(c, jlo, jhi):
                # contiguous plane-range transfer: big per-partition runs
                nc.sync.dma_start(
                    out=X[c][:, jlo * R : jhi * R], in_=x_d[c][:, jlo * R : jhi * R]
                )

            def exp(c, jlo, jhi):
                # whole contiguous plane range in one activation
                nc.scalar.activation(
                    E[c][:, jlo * R : jhi * R], X[c][:, jlo * R : jhi * R], Act.Exp
                )

            def exp_col(c, lo, hi):
                # column slice across all planes (strided view)
                n = 6 if c < 4 else 5
                xv = X[c].rearrange("p (j r) -> p j r", j=n)
                ev = E[c].rearrange("p (j r) -> p j r", j=n)
                nc.scalar.activation(ev[:, :, lo:hi], xv[:, :, lo:hi], Act.Exp)

            def adds_a1(c, lo=0, hi=None):  # GpSimd: planes 0,1 (+DVE max seg4)
                hi = R if hi is None else hi
                s = seg[c]
                e = lambda j: E[c][:, j * R + lo : j * R + hi]
                nc.gpsimd.tensor_tensor(
                    out=s["a1"][:, lo:hi], in0=e(0), in1=e(1), op=Alu.add
                )
                if c == 4:
                    # Pool engine has no max op — M-pairs run on DVE (2x bf16)
                    nc.vector.tensor_tensor(
                        out=s["m1"][:, lo:hi], in0=e(0), in1=e(1), op=Alu.max
                    )

            def adds_a2(c, lo=0, hi=None):  # GpSimd: planes 2,3
                hi = R if hi is None else hi
                s = seg[c]
                e = lambda j: E[c][:, j * R + lo : j * R + hi]
                nc.gpsimd.tensor_tensor(
                    out=s["a2"][:, lo:hi], in0=e(2), in1=e(3), op=Alu.add
                )
                if c == 4:
                    nc.vector.tensor_tensor(
                        out=s["m2"][:, lo:hi], in0=e(2), in1=e(3), op=Alu.max
                    )

            def adds_b(c, lo=0, hi=None):  # DVE combine stage
                if hi is None:
                    hi = R
                s = seg[c]
                e = lambda j: E[c][:, j * R + lo : j * R + hi]
                nc.vector.tensor_tensor(
                    out=s["a3"][:, lo:hi],
                    in0=s["a1"][:, lo:hi],
                    in1=s["a2"][:, lo:hi],
                    op=Alu.add,
                )
                nc.vector.tensor_tensor(
                    out=s["z"][:, lo:hi], in0=s["a3"][:, lo:hi], in1=e(4), op=Alu.add
                )
                if c == 4:
                    nc.vector.tensor_tensor(
                        out=s["m3"][:, lo:hi],
                        in0=s["m1"][:, lo:hi],
                        in1=s["m2"][:, lo:hi],
                        op=Alu.max,
                    )
                    nc.vector.tensor_tensor(
                        out=s["mx"][:, lo:hi],
                        in0=s["m3"][:, lo:hi],
                        in1=e(4),
                        op=Alu.max,
                    )

            def ln(c, lo, hi):
                s = seg[c]
                nc.scalar.activation(s["lnz"][:, lo:hi], s["z"][:, lo:hi], Act.Ln)

            def grp(c, lo, hi, part):
                # part in {0,1}: chunk-unique accumulator column
                s = seg[c]
                xp = lambda j: X[c][:, j * R + lo : j * R + hi]
                ep = lambda j: E[c][:, j * R + lo : j * R + hi]
                col = lambda k: stats[:, k : k + 1]
                w = lambda t: s[t][:, lo:hi]
                # d4 = lnZ - x4  (= -log p_neg)
                nc.vector.tensor_tensor(
                    out=w("d4"), in0=w("lnz"), in1=xp(4), op=Alu.subtract
                )
                if c < 4:
                    # sd_c = sum(x4) + sum(-x_c): planes 4,5 as one ts accum
                    if lo == 0 and hi == R:
                        x45 = X[c][:, 4 * R : 6 * R]
                        s45 = s["sc"]
                    else:
                        x45 = X[c].rearrange("p (j r) -> p j r", j=6)[:, 4:6, lo:hi]
                        s45 = s["sc"].rearrange("p (j r) -> p j r", j=2)[:, :, lo:hi]
                    nc.vector.tensor_scalar(
                        out=s45,
                        in0=x45,
                        scalar1=1.0,
                        scalar2=0.0,
                        op0=Alu.mult,
                        op1=Alu.add,
                        accum_out=col(8 * c + 0 + part),
                    )
                    # m = (2*E_c > Z), den_c = sum(m)
                    nc.vector.scalar_tensor_tensor(
                        out=w("m"),
                        in0=ep(c),
                        scalar=2.0,
                        in1=w("z"),
                        op0=Alu.mult,
                        op1=Alu.is_gt,
                        accum_out=col(8 * c + 2 + part),
                    )
                    # q = d4 * Z ; u = q * exp(-x_c)  (= -log(p4)/p_c)
                    nc.vector.tensor_tensor(
                        out=w("q"), in0=w("d4"), in1=w("z"), op=Alu.mult
                    )
                    nc.vector.tensor_tensor(
                        out=w("u"), in0=w("q"), in1=ep(5), op=Alu.mult
                    )
                    # num_c = sum(m * u)
                    nc.vector.scalar_tensor_tensor(
                        out=w("g"),
                        in0=w("m"),
                        scalar=1.0,
                        in1=w("u"),
                        op0=Alu.mult,
                        op1=Alu.mult,
                        accum_out=col(8 * c + 4 + part),
                    )
                else:
                    # mn = (2*MX <= Z) == all p <= 0.5
                    nc.vector.scalar_tensor_tensor(
                        out=w("m"),
                        in0=w("mx"),
                        scalar=2.0,
                        in1=w("z"),
                        op0=Alu.mult,
                        op1=Alu.is_le,
                        accum_out=None,
                    )
                    # li = sum(mn * d4)
                    nc.vector.scalar_tensor_tensor(
                        out=w("g"),
                        in0=w("m"),
                        scalar=1.0,
                        in1=w("d4"),
                        op0=Alu.mult,
                        op1=Alu.mult,
                        accum_out=col(32 + part),
                    )

            def alloc(c):
                n = 6 if c < 4 else 5
                X[c] = iop.tile([P, n * R], BF16, tag=f"x{c}", name=f"x{c}")
                E[c] = epool.tile([P, n * R], BF16, tag=f"e{c}", name=f"e{c}")
                s = {}
                names = ("a1", "a2", "a3", "z", "lnz", "d4", "m", "g")
                if c < 4:
                    names += ("q", "u")
                else:
                    names += ("m1", "m2", "m3", "mx")
                for t in names:
                    s[t] = wp.tile([P, R], BF16, tag=f"{t}_{c}", name=f"{t}_{c}")
                if c < 4:
                    s["sc"] = wp.tile([P, 2 * R], BF16, tag=f"sc_{c}", name=f"sc_{c}")
                seg[c] = s

            for c in range(NCLS):
                alloc(c)
            h = R // 2
            # DMAs: one hardware queue drains in issue order; contiguous
            # plane-range transfers keep per-partition runs >= 4.7KB.
            dma_in(4, 0, 3)
            dma_in(4, 3, 5)
            dma_in(0, 0, 3)
            dma_in(0, 3, 6)
            dma_in(1, 0, 6)
            dma_in(2, 0, 6)
            dma_in(3, 0, 6)
            # software-pipelined emission; per-engine in-order streams matter
            exp(4, 0, 3)
            adds_a1(4)
            exp(4, 3, 5)
            adds_a2(4)
            adds_b(4)
            exp(0, 0, 3)
            adds_a1(0)
            exp(0, 3, 6)
            ln(4, 0, R)
            adds_a2(0)
            adds_b(0)
            exp(1, 0, 6)
            ln(0, 0, R)
            grp(4, 0, R, 0)
            adds_a1(1)
            adds_a2(1)
            adds_b(1)
            exp(2, 0, 6)
            ln(1, 0, R)
            grp(0, 0, R, 0)
            adds_a1(2)
            adds_a2(2)
            adds_b(2)
            exp_col(3, 0, h)
            ln(2, 0, R)
            grp(1, 0, R, 0)
            adds_a1(3, 0, h)
            adds_a2(3, 0, h)
            adds_b(3, 0, h)
            exp_col(3, h, R)
            ln(3, 0, h)
            grp(2, 0, R, 0)
            adds_a1(3, h, R)
            adds_a2(3, h, R)
            adds_b(3, h, R)
            ln(3, h, R)
            grp(3, 0, h, 0)
            grp(3, h, R, 1)
            nc.sync.dma_start(out=st_d, in_=stats)
    nc.compile()
    return nc


def _get_program(R: int):
    if R not in _PROGRAM_CACHE:
        _PROGRAM_CACHE[R] = _build_program(R)
    return _PROGRAM_CACHE[R]


def _prepare_inputs(x: np.ndarray, t: np.ndarray):
    """Sort rows by class, shard across cores, pad segments, pack planar bf16.
    Returns (in_maps, counts, n_pad_per_class_total, R)."""
    N = x.shape[0]
    t64 = t.astype(np.int64, copy=False)
    counts = np.bincount(t64, minlength=NCLS).astype(np.int64)

    # per-core per-class row counts (even split of each class across cores)
    n_ck = np.zeros((NCLS, N_CORES), dtype=np.int64)
    for c in range(NCLS):
        q, r = divmod(int(counts[c]), N_CORES)
        n_ck[c] = q
        n_ck[c, :r] += 1

    R = int(max(8, -(-int(n_ck.max()) // P)))
    R = (R + 1) // 2 * 2  # keep it even
    S = P * R

    order = np.argsort(t64, kind="stable")
    xs = np.ascontiguousarray(x[order], dtype=np.float32)
    starts = np.concatenate([[0], np.cumsum(counts)])

    # planar layout per (core, segment): [P, 6 planes, R]; plane 5 is a hole
    xcores = np.zeros((N_CORES, NCLS, P, 6, R), dtype=np.float32)
    xcores[:, :, :, :4, :] = PAD_POS
    xcores[:, :, :, 4, :] = PAD_NEG
    xcores[:, :, :, 5, :] = -PAD_POS  # -x_c for pad rows (c < 4)
    for c in range(NCLS):
        off = int(starts[c])
        for k in range(N_CORES):
            n = int(n_ck[c, k])
            if n:
                blk = np.empty((S, 5), dtype=np.float32)
                blk[:n] = xs[off : off + n]
                blk[n:, :4] = PAD_POS
                blk[n:, 4] = PAD_NEG
                # row i -> (p, r) = (i // R, i % R); planes transposed in
                pl = blk.reshape(P, R, 5).transpose(0, 2, 1)
                xcores[k, c, :, :5, :] = pl
                xcores[k, c, :, 5, :] = -pl[:, c, :]
                off += n

    xb = xcores.reshape(N_CORES, NCLS, P, 6 * R).astype(ml_dtypes.bfloat16)
    in_maps = [{"x": xb[k]} for k in range(N_CORES)]
    n_pad = N_CORES * S - counts  # per class, summed over cores
    return in_maps, counts, n_pad, R


def _combine(stats_list, counts, n_pad, N, R):
    """Host all-reduce of the C-length accumulators + final scalar combination."""
    st = np.zeros(NSTAT, dtype=np.float64)
    for s in stats_list:
        st += s.astype(np.float64).sum(axis=0)

    counts = counts.astype(np.float64)
    r13 = 0.0  # risk1 - risk3
    r2 = 0.0
    for c in range(4):
        sd = st[8 * c + 0] + st[8 * c + 1]
        den = st[8 * c + 2] + st[8 * c + 3]
        num = st[8 * c + 4] + st[8 * c + 5]
        sd -= 20.0 * float(n_pad[c])  # sum_{t=c}(x4 - xc), pads removed
        prior = counts[c] / N
        r13 += prior * sd / max(1.0, counts[c])
        r2 += prior * num / max(den, 1.0)
    li = st[32] + st[33]
    r4 = li / max(1.0, counts[4])

    pos = 4.0 * (r13 + r2)
    if pos < 0.0:
        pos = 0.0
    return np.float32(pos + r4)


def run_device(in_maps, R, trace=False, **kw):
    nc = _get_program(R)
    res = bass_utils.run_bass_kernel_spmd(
        nc, in_maps, core_ids=list(range(N_CORES)), trace=trace, **kw
    )
    return res


def kernel(x: np.ndarray, t: np.ndarray) -> np.ndarray:
    x = np.asarray(x, dtype=np.float32)
    t = np.asarray(t)
    N = x.shape[0]
    in_maps, counts, n_pad, R = _prepare_inputs(x, t)
    res = run_device(in_maps, R)
    stats_list = [res.results[k]["stats"] for k in range(N_CORES)]
    return _combine(stats_list, counts, n_pad, N, R)
